# revision 1
# baseline (speedup 1.0000x reference)
"""Trainium2 Bass kernel for nn_ChannelSA3dCausal.

Computation (per batch b, time t):
  framed[c, t, d] = xpad[c, t+d]            (causal window, D=32)
  q/k = relu(BN(framed @ W^T + b))          (1x1 conv D->S=32, BN folded)
  ssa[i, j] = sum_s q[i,s] k[j,s] / sqrt(S)
  att[i] = sum_j softmax_j(ssa)[i,j] * x[j]
  out = x + att

Sharding: 8 cores = 4 batches x 2 time-halves (512 t each + 31-frame halo,
host-padded). Full inputs in, full output out.

Per-core pipeline (ScalarE-bound: 33.5M exp evals):
  - 4 "phases" g handle t = 128*g + tl; quad(tl) = 4 t's processed together.
  - framed SBUF tile [128, (c,tl)]: partition 32g+d holds x[c, 128g+tl+d] windows.
  - proj: ONE full-K matmul per q/k with host-built block-diagonal weights
    [128,128] -> all 4 phases at once; + fused DVE bias+relu evac -> fp32r.
  - ssa^T: per phase, 2 row-tiled fp32r matmuls (K=32 at tile_position (32g,0)),
    each concurrent tile in its own PSUM bank -> psum [j, i] per t.
  - exp: ACT FD=1024 (2 banks = 2 phases), scale=1/sqrt(32) folded in, out bf16.
    ReLU => ssa >= 0 => no max-subtraction needed (max scaled ssa ~17 << 88).
  - num/den: E^T as bf16 stationary [128,128], rhs = [x_t | 1] interleaved
    (host-built) -> psum [i, (num,den)] accumulating 128 t per bank.
  - divide + add x on DVE in [channel, t] layout; single output DMA.
"""

import sys

sys.path.insert(0, "/opt/trn_rl_repo")

import numpy as np

_KERNEL_CACHE = {}

B, C, T, D = 4, 256, 1024, 32
S = D
BN_EPS = 1e-5
TCORE = T // 2  # 512 t per core
HALO = D - 1  # 31
XSL_W = TCORE + HALO  # 543
NPH = 4  # phases
TPH = TCORE // NPH  # 128 t per phase
TBLK = 32  # tl per framed/nd block
NBLK = TPH // TBLK  # 4


def _build_program(reps=1):
    import concourse.bass as bass
    import concourse.bacc as bacc
    import concourse.tile as tile
    from concourse import mybir

    FP32 = mybir.dt.float32
    FP32R = mybir.dt.float32r
    BF16 = mybir.dt.bfloat16
    AF = mybir.ActivationFunctionType
    ADD = mybir.AluOpType.add
    MAX = mybir.AluOpType.max

    nc = bacc.Bacc("TRN2", target_bir_lowering=False, debug=False)

    xsl_d = nc.dram_tensor("xsl", [C, XSL_W], FP32, kind="ExternalInput")
    xslr_d = nc.dram_tensor("xslr", [C, XSL_W], FP32R, kind="ExternalInput")
    wblk_d = nc.dram_tensor("wblk", [128, 256], FP32R, kind="ExternalInput")
    bias_d = nc.dram_tensor("bias", [128, 2], FP32, kind="ExternalInput")
    xot_d = nc.dram_tensor("xot", [128, 4 * TCORE], BF16, kind="ExternalInput")
    out_d = nc.dram_tensor("out", [C, TCORE], FP32, kind="ExternalOutput")

    scale = float(1.0 / np.sqrt(np.float32(S)))

    with tile.TileContext(nc) as tc:
        with (
            tc.tile_pool(name="persist", bufs=1) as pers,
            tc.tile_pool(name="framedp", bufs=2) as framedp,
            tc.tile_pool(name="qkp", bufs=3) as qkp,
            tc.tile_pool(name="ep", bufs=2) as ep,
            tc.tile_pool(name="divp", bufs=2) as divp,
            tc.tile_pool(name="pp", bufs=2, space="PSUM") as pp,
            tc.tile_pool(name="sp", bufs=1, space="PSUM") as sp,
            tc.tile_pool(name="ndp", bufs=2, space="PSUM") as ndp,
        ):
            # persistent loads
            xc = pers.tile([128, 2 * XSL_W], FP32, tag="xc")
            xc_dst = bass.AP(
                tensor=xc[:].tensor,
                offset=xc[:].offset,
                ap=[[2 * XSL_W, 128], [XSL_W, 2], [1, XSL_W]],
            )
            nc.sync.dma_start(
                xc_dst,
                bass.AP(
                    tensor=xsl_d,
                    offset=0,
                    ap=[[XSL_W, 128], [128 * XSL_W, 2], [1, XSL_W]],
                ),
            )
            wblk = pers.tile([128, 256], FP32R, tag="wblk")
            nc.sync.dma_start(wblk[:], wblk_d.ap())
            biases = pers.tile([128, 2], FP32, tag="bias")
            nc.sync.dma_start(biases[:], bias_d.ap())
            xot = pers.tile([128, 4 * TCORE], BF16, tag="xot")
            nc.sync.dma_start(xot[:], xot_d.ap())
            out_sb = pers.tile([128, 2 * TCORE], FP32, tag="osb")

            for _rep in range(reps):
              for blk in range(NBLK):
                  # framed tile: partition 32g+d, free col = c*TBLK + tls
                  # value = xsl[c, 128*g + TBLK*blk + tls + d]
                  # phase g covers t = 128*blk + 32*g + tls, so partition
                  # p = 32g+d maps to x-offset p + tls + 128*blk (affine in p).
                  framed = framedp.tile([128, C * TBLK], FP32R, tag="framed")
                  fr_dst = bass.AP(
                      tensor=framed[:].tensor,
                      offset=framed[:].offset,
                      ap=[[C * TBLK, 128], [TBLK, C], [1, TBLK]],
                  )
                  fr_src = bass.AP(
                      tensor=xslr_d,
                      offset=TPH * blk,
                      ap=[[1, 128], [XSL_W, C], [1, TBLK]],
                  )
                  nc.sync.dma_start(fr_dst, fr_src)

                  ndt = ndp.tile([128, 512], FP32, tag="ndt")

                  for tls in range(TBLK):
                      tl = TBLK * blk + tls
                      # --- proj: block-diag full-K matmuls (q, k) ---
                      prp = pp.tile([128, 512], FP32, tag="prp")
                      rhs = bass.AP(
                          tensor=framed[:].tensor,
                          offset=framed[:].offset + tls,
                          ap=[[C * TBLK, 128], [TBLK, C]],
                      )
                      nc.tensor.matmul(
                          prp[:, 0:256], wblk[:, 0:128], rhs, start=True, stop=True
                      )
                      nc.tensor.matmul(
                          prp[:, 256:512], wblk[:, 128:256], rhs, start=True, stop=True
                      )
                      # --- evac: relu(h + bias) -> fp32r ---
                      q_t = qkp.tile([128, 256], FP32R, tag="qt")
                      nc.vector.tensor_scalar(
                          q_t[:], prp[:, 0:256], biases[:, 0:1], 0.0, op0=ADD, op1=MAX
                      )
                      k_t = qkp.tile([128, 256], FP32R, tag="kt")
                      nc.vector.tensor_scalar(
                          k_t[:], prp[:, 256:512], biases[:, 1:2], 0.0, op0=ADD, op1=MAX
                      )
                      # --- ssa^T: row-tiled fp32r, phase g -> own bank ---
                      eA = sp.tile([128, 1024], FP32, tag="eA")
                      eB = sp.tile([128, 1024], FP32, tag="eB")
                      for g in range(NPH):
                          dst = eA if g < 2 else eB
                          for jc in range(2):
                              nc.tensor.matmul(
                                  dst[
                                      :,
                                      512 * (g % 2) + 256 * jc : 512 * (g % 2)
                                      + 256 * (jc + 1),
                                  ],
                                  k_t[32 * g : 32 * g + 32, 128 * jc : 128 * (jc + 1)],
                                  q_t[32 * g : 32 * g + 32, :],
                                  start=True,
                                  stop=True,
                                  tile_position=(32 * g, 0),
                              )
                      # --- exp -> bf16, nd interleaved per phase-pair ---
                      EA = ep.tile([128, 1024], BF16, tag="EA")
                      EB = ep.tile([128, 1024], BF16, tag="EB")
                      nc.scalar.activation(EA[:], eA[:], AF.Exp, scale=scale)
                      nc.scalar.activation(EB[:], eB[:], AF.Exp, scale=scale)
                      # --- num/den: E^T stationary bf16 ---
                      for g in (0, 1, 2, 3):
                          E_ = EA if g < 2 else EB
                          tp = TPH * blk + TBLK * g + tls  # t within core
                          colb = 16 * tls + 4 * g
                          for ic in range(2):
                              for jc in range(2):
                                  nc.tensor.matmul(
                                      ndt[:, colb + 2 * ic : colb + 2 * ic + 2],
                                      E_[
                                          :,
                                          512 * (g % 2)
                                          + 256 * jc
                                          + 128 * ic : 512 * (g % 2)
                                          + 256 * jc
                                          + 128 * (ic + 1),
                                      ],
                                      xot[:, 2 * TCORE * jc + 2 * tp : 2 * TCORE * jc + 2 * tp + 2],
                                      start=(jc == 0),
                                      stop=(jc == 1),
                                  )

                  # --- block epilogue: evac nd, divide, add x ---
                  nd_sb = divp.tile([128, 512], FP32, tag="ndsb")
                  nc.vector.tensor_copy(nd_sb[:], ndt[:])
                  # cols: 16*tls + 4*g + 2*ic + e   (e: 0=num, 1=den)
                  rden = divp.tile([128, 256], FP32, tag="rden")
                  rden4 = bass.AP(
                      tensor=rden[:].tensor,
                      offset=rden[:].offset,
                      ap=[[256, 128], [8, TBLK], [2, NPH], [1, 2]],
                  )
                  den_ap = bass.AP(
                      tensor=nd_sb[:].tensor,
                      offset=nd_sb[:].offset + 1,
                      ap=[[512, 128], [16, TBLK], [4, NPH], [2, 2]],
                  )
                  nc.vector.reciprocal(rden4, den_ap)
                  att = divp.tile([128, 256], FP32, tag="att")
                  att4 = bass.AP(
                      tensor=att[:].tensor,
                      offset=att[:].offset,
                      ap=[[256, 128], [8, TBLK], [2, NPH], [1, 2]],
                  )
                  num_ap = bass.AP(
                      tensor=nd_sb[:].tensor,
                      offset=nd_sb[:].offset,
                      ap=[[512, 128], [16, TBLK], [4, NPH], [2, 2]],
                  )
                  nc.vector.tensor_tensor(att4, num_ap, rden4, op=mybir.AluOpType.mult)
                  # out_sb[p, ic*TCORE + 128*g + TBLK*blk + tls] = att + x
                  out_ap = bass.AP(
                      tensor=out_sb[:].tensor,
                      offset=out_sb[:].offset + TPH * blk,
                      ap=[[2 * TCORE, 128], [1, TBLK], [TBLK, NPH], [TCORE, 2]],
                  )
                  x_ap = bass.AP(
                      tensor=xc[:].tensor,
                      offset=xc[:].offset + HALO + TPH * blk,
                      ap=[[2 * XSL_W, 128], [1, TBLK], [TBLK, NPH], [XSL_W, 2]],
                  )
                  nc.vector.tensor_tensor(out_ap, att4, x_ap, op=ADD)

              # final store: out[ch*128+p, t'] = out_sb[p, ch*TCORE + t']
              osb_src = bass.AP(
                  tensor=out_sb[:].tensor,
                  offset=out_sb[:].offset,
                  ap=[[2 * TCORE, 128], [TCORE, 2], [1, TCORE]],
              )
              nc.sync.dma_start(
                  bass.AP(
                      tensor=out_d,
                      offset=0,
                      ap=[[TCORE, 128], [128 * TCORE, 2], [1, TCORE]],
                  ),
                  osb_src,
              )

    nc.compile()
    return nc


def _host_prep(inputs):
    """Fold BN into weights, build per-core input maps."""
    x = np.asarray(inputs["x"], dtype=np.float32)  # [B, C, T, 1]
    xs = x[..., 0]  # [B, C, T]

    def fold(w, b, gamma, beta):
        g = np.asarray(gamma, np.float32) / np.sqrt(np.float32(1.0 + BN_EPS))
        wp = np.asarray(w, np.float32) * g[:, None]  # [s, d]
        bp = np.asarray(b, np.float32) * g + np.asarray(beta, np.float32)
        return wp, bp

    def round_fp32r(a):
        # approximate the fp32r operand rounding (~13-14 mantissa bits kept);
        # the PE truncates further internally either way.
        u = np.ascontiguousarray(a, np.float32).view(np.uint32)
        u = (u + np.uint32(0x100)) & np.uint32(0xFFFFFE00)
        return u.view(np.float32)

    wq, bq = fold(
        inputs["query_w"], inputs["query_b"], inputs["query_gamma"], inputs["query_beta"]
    )
    wk, bk = fold(
        inputs["key_w"], inputs["key_b"], inputs["key_gamma"], inputs["key_beta"]
    )

    # block-diag weights [128, 256]: [:, 0:128]=q, [:, 128:256]=k
    # wblk[32g+d, 32g+s] = w[s, d]
    wblk = np.zeros((128, 256), np.float32)
    for g in range(NPH):
        wblk[32 * g : 32 * g + 32, 32 * g : 32 * g + 32] = wq.T
        wblk[32 * g : 32 * g + 32, 128 + 32 * g : 128 + 32 * g + 32] = wk.T
    bias2 = np.zeros((128, 2), np.float32)
    bias2[:, 0] = np.tile(bq, NPH)
    bias2[:, 1] = np.tile(bk, NPH)

    xpad = np.concatenate([np.zeros((B, C, HALO), np.float32), xs], axis=2)

    import ml_dtypes

    in_maps = []
    for core in range(8):
        b, th = core // 2, core % 2
        t0 = th * TCORE
        xsl = np.ascontiguousarray(xpad[b, :, t0 : t0 + XSL_W])  # [C, 543]
        xslr = round_fp32r(xsl)
        # xot [128, 4*TCORE] bf16: [p, 2*TCORE*jc + 2*t' + e]
        xot = np.ones((128, 4 * TCORE), np.float32)
        for jc in range(2):
            xot[:, 2 * TCORE * jc : 2 * TCORE * (jc + 1) : 2] = xs[
                b, 128 * jc : 128 * (jc + 1), t0 : t0 + TCORE
            ]
        in_maps.append(
            {
                "xsl": xsl,
                "xslr": xslr,
                "wblk": round_fp32r(wblk),
                "bias": bias2,
                "xot": xot.astype(ml_dtypes.bfloat16),
            }
        )
    return in_maps


def kernel(**inputs):
    from concourse.bass_utils import run_bass_kernel_spmd

    if "nc" not in _KERNEL_CACHE:
        _KERNEL_CACHE["nc"] = _build_program()
    nc = _KERNEL_CACHE["nc"]

    in_maps = _host_prep(inputs)
    res = run_bass_kernel_spmd(nc, in_maps, core_ids=list(range(8)))
    _KERNEL_CACHE["last_results"] = res

    x = np.asarray(inputs["x"], dtype=np.float32)
    out = np.empty((B, C, T, 1), dtype=np.float32)
    for core in range(8):
        b, th = core // 2, core % 2
        t0 = th * TCORE
        out[b, :, t0 : t0 + TCORE, 0] = res.results[core]["out"]
    return out

